# revision 62
# baseline (speedup 1.0000x reference)
"""Grouped-experts MLP (MoE) kernel for Trainium2, expert-parallel over 8 cores.

Problem: x[B=2, E=8, N=1024, D=1024]; per expert e:
    out[:, e] = GELU(x[:, e] @ w1[e] + b1[e]) @ w2[e] + b2[e]
with w1[e]: [D=1024, H=4096], w2[e]: [H=4096, D=1024].

Sharding: expert axis across the 8 NeuronCores (core e owns expert e).
The host performs the "all-to-all": it hands core e the slab x[:, e]
plus expert e's weights and reassembles the full output afterward.

Per-core kernel (T = B*N = 2048 tokens), all matmul operands bf16 with
fp32 PSUM accumulation. The PE floor is 2048 matmuls x 512 cols
(~443us at ~2.4GHz); everything else is startup/tail/bubble control:

  - All inputs are host-packed into SBUF-image DRAM tensors [128, N]
    so every DMA is one large contiguous 2D slice (2KB+ rows nearly
    double aggregate DMA bandwidth vs 1KB rows). DMA completion
    follows issue order, and early aggregate bandwidth ramps from
    ~100 to ~400GB/s over the first ~8us - that ramp, not issue
    serialization, floors how fast the first MB can land.
  - To fill the ramp window the host also passes a duplicated
    256-token "head" slab of x-q0 (256KB). It and w1h0 land ~12.5us
    (the ramp floor), ~1-3us before the full 1MB x-q0, so the first 6
    layer-1 h-groups run as 256-column slab matmuls on the head
    tokens; their other halves follow immediately after (reusing
    w1h0-5, giving the w1 stream catch-up time before h6+).
  - An 11-matmul warm-up block of garbage matmuls spans the framework
    preamble end (~7.8us) to the first data (~12.5us). The HAM clock
    governor needs ~3.5us of GAPLESS PE activity to un-throttle
    1.2->2.4GHz (any sub-us stall resets the ramp while throttled;
    once full speed, brief stalls are safe - only multi-us idle
    re-throttles), so the warm block both finishes the clock ramp and
    bridges to the data with no gap.
  - w1 is packed h-major so each 256KB DMA feeds exactly one layer-1
    h-group; sync streams them in consumption order, then x-q1, w2,
    b2. Layer 1 computes hT[h][128, 512] per h-group (PSUM accum over
    8 D-tiles), GELU + b1 fused into the PSUM->SBUF eviction on the
    scalar engine, writing bf16.
  - Layer 2 accumulates 32 H-tiles of a [128 tok, 512 dcol] tile in
    PSUM, adds b2 on the DVE, DMAs out. The very last group runs as
    two 256-col accumulation sub-groups in separate PSUM tiles
    (sharing one tile makes B's start falsely wait on A's DVE read):
    half A's eviction + DMA overlap half B's matmuls, so after the
    last matmul only a 256-col DVE add and two parallel 32KB DMAs
    (sync + scalar; gpsimd's SWDGE drain is ~2.6us - never on the
    tail) remain before the fixed ~2.5us teardown. The output is
    written bf16 (host upcasts): halves out-DMA bytes and doubles the
    DVE eviction rate for ~1e-3 extra rel-err.
"""

import numpy as np
import ml_dtypes

import concourse.bacc as bacc
import concourse.mybir as mybir
import concourse.tile as tile
from concourse.bass_utils import run_bass_kernel_spmd

B, E, N, D, H = 2, 8, 1024, 1024, 4096
T = B * N          # tokens per expert
P = 128
N_CORES = 8

TQ = 512           # tokens per quarter
NQ = T // TQ       # 4
KD = D // P        # 8 k-tiles over D (layer-1 contraction)
KH = H // P        # 32 k-tiles over H (layer-2 contraction)
DC = D // 512      # 2 output column chunks
NTS = TQ // P      # 4 token subtiles per quarter
NSLAB = 6          # leading q0 h-groups computed as 256-token half slabs

F32 = mybir.dt.float32
BF16 = mybir.dt.bfloat16
GELU = mybir.ActivationFunctionType.Gelu
BF16_NP = ml_dtypes.bfloat16


def build_nc():
    nc = bacc.Bacc("TRN2", target_bir_lowering=False, debug=False)

    # Host-packed SBUF images: [128, cols]; column layout is the exact
    # SBUF layout, so each DMA is a contiguous 2D slice.
    #   xin:   [q][k][512]  (q*4096 + k*512)
    #   xhead: [k][256]     q0 tokens 0:255 duplicated (fast first slab)
    #   w1in:  [h][k][128]  (h*1024 + k*128)   h-major: DMA unit = h-group
    #   w2in:  [dc][k][512] (dc*16384 + k*512)
    xin = nc.dram_tensor("xin", [P, NQ * KD * TQ], BF16,
                         kind="ExternalInput")          # [128, 16384]
    xhead = nc.dram_tensor("xhead", [P, KD * 256], BF16,
                           kind="ExternalInput")        # [128, 2048]
    w1in = nc.dram_tensor("w1in", [P, D * H // P], BF16,
                          kind="ExternalInput")         # [128, 32768]
    w2in = nc.dram_tensor("w2in", [P, H * D // P], BF16,
                          kind="ExternalInput")         # [128, 32768]
    b1 = nc.dram_tensor("b1", [P, KH], F32, kind="ExternalInput")
    b2 = nc.dram_tensor("b2", [P, D], F32, kind="ExternalInput")
    # output in bf16: halves out-DMA bytes, doubles DVE eviction rate,
    # and shrinks the tail transfers; host upcasts to fp32 (adds at most
    # ~1.1e-3 to the rel-err, far inside the 2e-2 gate)
    out = nc.dram_tensor("out", [T, D], BF16, kind="ExternalOutput")
    scratch = nc.dram_tensor("scratch", [P, 8], BF16, kind="Internal")

    with tile.TileContext(nc) as tc:
        with (
            tc.tile_pool(name="const", bufs=1) as constp,
            tc.tile_pool(name="xp", bufs=2) as xp,
            tc.tile_pool(name="w1p", bufs=1) as w1p,
            tc.tile_pool(name="w2p", bufs=1) as w2p,
            tc.tile_pool(name="hTp", bufs=1) as hTp,
            tc.tile_pool(name="stp", bufs=4) as stp,
            tc.tile_pool(name="ps1p", bufs=5, space="PSUM") as ps1p,
            tc.tile_pool(name="ps2p", bufs=3, space="PSUM") as ps2p,
        ):
            def alloc_xq():
                return xp.tile([P, 4096], BF16, name="xq", tag="xq")

            def load_xq(eng, t, q):
                eng.dma_start(t[:], xin[:, q * 4096:(q + 1) * 4096])

            b1sb = constp.tile([P, KH], F32, name="b1sb")
            warm_src = constp.tile([P, 512], BF16, name="warm_src")
            xh = constp.tile([P, KD * 256], BF16, name="xh")
            xq0 = constp.tile([P, 4096], BF16, name="xq0")
            xq_tiles = [None] * NQ
            w1h = [w1p.tile([P, KD * P], BF16, name=f"w1h_{h}")
                   for h in range(KH)]

            def w1_st(h, k):
                return w1h[h][:, k * P:(k + 1) * P]

            def xh_mov(k):
                return xh[:, k * 256:(k + 1) * 256]

            # ---- startup DMAs, in consumption order across engines ----
            # Pre-gate traffic is exactly what the slab-A phase needs
            # (xh + w1h0-4 + b1, ~2.1MB). Everything else (xq0 first,
            # then the w1h stream, x-q1, w2, b2) waits on a gate DMA
            # that reads the first slab eviction, so it cannot crowd
            # the DMA FIFO during the bandwidth ramp. The gate must NOT
            # sit on scalar: it would deadlock against the ACTIVATE it
            # waits for.
            nc.sync.dma_start(w1h[0][:], w1in[:, 0:1024])
            nc.scalar.dma_start(xh[:], xhead[:])
            nc.gpsimd.memset(warm_src[:], 0.0)
            nc.gpsimd.dma_start(w1h[1][:], w1in[:, 1024:2048])
            nc.gpsimd.dma_start(b1sb[:], b1[:])
            nc.sync.dma_start(w1h[2][:], w1in[:, 2 * 1024:3 * 1024])
            nc.sync.dma_start(w1h[3][:], w1in[:, 3 * 1024:4 * 1024])
            nc.sync.dma_start(w1h[4][:], w1in[:, 4 * 1024:5 * 1024])
            nc.sync.dma_start(w1h[5][:], w1in[:, 5 * 1024:6 * 1024])
            w2t = {}
            b2sb = constp.tile([P, D], F32, name="b2sb")

            def emit_gated_loads(gate_tile):
                nc.sync.dma_start(scratch[:], gate_tile[:, 0:8])
                # xq0 is the ONLY post-gate transfer in flight for its
                # first ~2.6us (w1h6+ queue behind it), so it lands
                # ~gate+3us — before the slab-B phase needs it
                nc.sync.dma_start(xq0[:], xin[:, 0:4096])
                for h in range(6, KH):
                    nc.sync.dma_start(
                        w1h[h][:], w1in[:, h * 1024:(h + 1) * 1024])
                xq_tiles[1] = alloc_xq()
                load_xq(nc.sync, xq_tiles[1], 1)
                # w2: 2 dc-halves, 4 tiles [128,4096] each (k-octet)
                for dc in range(DC):
                    for i in range(4):
                        t = w2p.tile([P, 4096], BF16, name=f"w2_{dc}_{i}")
                        nc.sync.dma_start(
                            t[:], w2in[:, dc * 16384 + i * 4096:
                                       dc * 16384 + (i + 1) * 4096])
                        w2t[(dc, i)] = t
                nc.sync.dma_start(b2sb[:], b2[:])

            # ---- HAM pre-warm ----
            warm_ps = ps2p.tile([P, 512], F32, name="warm_ps", tag="ps2")
            NWARM = 14
            for i in range(NWARM):
                nc.tensor.matmul(
                    warm_ps[:], warm_src[:, 0:P], warm_src[:],
                    start=(i == 0), stop=(i == NWARM - 1))

            def x_ap(q, xq, k):
                if q == 0:
                    return xq0[:, k * 512:(k + 1) * 512]
                return xq[:, k * 512:(k + 1) * 512]

            def w2_ap(dc, k):
                return w2t[(dc, k // 8)][:, (k % 8) * 512:(k % 8) * 512 + 512]

            def alloc_ht(h):
                return hTp.tile([P, TQ], BF16, name=f"hT_{h}", tag=f"hT_{h}")

            def l1_group(h, mov, ht, ht_sl):
                """One layer-1 h-group: 8 matmuls + fused GELU eviction.
                mov(k) gives the moving operand; ht_sl the hT col range."""
                w = ht_sl.stop - ht_sl.start
                ps = ps1p.tile([P, TQ], F32, name="ps1", tag="ps1")
                for k in range(KD):
                    nc.tensor.matmul(
                        ps[:, 0:w], w1_st(h, k), mov(k),
                        start=(k == 0), stop=(k == KD - 1))
                nc.scalar.activation(
                    ht[:, ht_sl], ps[:, 0:w], GELU, bias=b1sb[:, h:h + 1])

            for q in range(NQ):
                xq = xq_tiles[q]
                hTt = [None] * KH

                if q == 0:
                    # leading h-groups on the 256-token head slab (xh
                    # lands ~3us before the full x-q0)...
                    for h in range(NSLAB):
                        hTt[h] = alloc_ht(h)
                        l1_group(h, xh_mov, hTt[h], slice(0, 256))
                        if h == 0:
                            emit_gated_loads(hTt[0])
                    # ...then their trailing halves once x-q0 has landed
                    # (these reuse w1h0-5, giving sync's w1h stream ~5us
                    # of catch-up before h6+ needs fresh tiles)...
                    for h in range(NSLAB):
                        l1_group(
                            h, lambda k: xq0[:, k * 512 + 256:
                                             (k + 1) * 512],
                            hTt[h], slice(256, TQ))
                    # ...and the remaining full-width groups.
                    for h in range(NSLAB, KH):
                        hTt[h] = alloc_ht(h)
                        l1_group(
                            h, lambda k: x_ap(q, xq, k), hTt[h],
                            slice(0, TQ))
                else:
                    for h in range(KH):
                        hTt[h] = alloc_ht(h)
                        l1_group(
                            h, lambda k: x_ap(q, xq, k), hTt[h],
                            slice(0, TQ))

                # prefetch x for quarter q+2 (q/q+1 tiles both live;
                # pool bufs=2 recycles q's buffers once layer 1 is done)
                if q + 2 < NQ:
                    xq_tiles[q + 2] = alloc_xq()
                    load_xq(nc.sync, xq_tiles[q + 2], q + 2)

                # layer 2: out tile [128 tok, 512 dcol] accumulates all 32
                # H-tiles in PSUM, then +b2 on the DVE and straight to DRAM
                for dc in range(DC):
                    sl = slice(dc * 512, (dc + 1) * 512)
                    for ts in range(NTS):
                        t0 = q * TQ + ts * P
                        ps = ps2p.tile([P, 512], F32, name="ps2", tag="ps2")
                        last = (q == NQ - 1 and dc == DC - 1 and ts == NTS - 1)
                        if not last:
                            for k in range(KH):
                                nc.tensor.matmul(
                                    ps[:], hTt[k][:, ts * P:(ts + 1) * P],
                                    w2_ap(dc, k),
                                    start=(k == 0), stop=(k == KH - 1))
                            st = stp.tile([P, 512], BF16, name="st", tag="st")
                            nc.vector.tensor_add(st[:], b2sb[:, sl], ps[:])
                            nc.sync.dma_start(out[t0:t0 + P, sl], st[:])
                        else:
                            # tail: two 256-col sub-groups, separate PSUM
                            psB = ps2p.tile([P, 512], F32, name="ps2b",
                                            tag="ps2")
                            for hf in range(2):
                                pst = ps if hf == 0 else psB
                                c0 = dc * 512 + hf * 256
                                for k in range(KH):
                                    w2a = w2t[(dc, k // 8)][
                                        :, (k % 8) * 512 + hf * 256:
                                        (k % 8) * 512 + hf * 256 + 256]
                                    nc.tensor.matmul(
                                        pst[:, 0:256],
                                        hTt[k][:, ts * P:(ts + 1) * P],
                                        w2a,
                                        start=(k == 0), stop=(k == KH - 1))
                                stf = constp.tile([P, 256], BF16,
                                                  name=f"stf_{hf}")
                                nc.vector.tensor_add(
                                    stf[:], b2sb[:, c0:c0 + 256],
                                    pst[:, 0:256])
                                if hf == 0:
                                    nc.sync.dma_start(
                                        out[t0:t0 + P, c0:c0 + 256], stf[:])
                                else:
                                    nc.sync.dma_start(
                                        out[t0:t0 + P, c0:c0 + P],
                                        stf[:, 0:P])
                                    nc.scalar.dma_start(
                                        out[t0:t0 + P, c0 + P:c0 + 256],
                                        stf[:, P:256])

    nc.compile()
    return nc


def make_in_map(x_e, w1_e, b1_e, w2_e, b2_e):
    """Per-core input map: pack one expert's slabs into SBUF images."""
    xT = x_e.reshape(T, D).T.astype(BF16_NP)      # [D, T]
    xin = np.ascontiguousarray(
        xT.reshape(KD, P, NQ, TQ).transpose(1, 2, 0, 3).reshape(P, -1)
    )                                             # [128, q*4096 + k*512]
    xhead = np.ascontiguousarray(
        xT[:, 0:256].reshape(KD, P, 256).transpose(1, 0, 2).reshape(P, -1)
    )                                             # [128, k*256]
    w1in = np.ascontiguousarray(
        w1_e.reshape(KD, P, KH, P).transpose(1, 2, 0, 3).reshape(P, -1)
    ).astype(BF16_NP)                             # [128, h*1024 + k*128]
    w2in = np.ascontiguousarray(
        w2_e.reshape(KH, P, DC, 512).transpose(1, 2, 0, 3).reshape(P, -1)
    ).astype(BF16_NP)                             # [128, dc*16384 + k*512]
    return {
        "xin": xin,
        "xhead": xhead,
        "w1in": w1in,
        "w2in": w2in,
        "b1": np.ascontiguousarray(b1_e.reshape(KH, P).T),
        "b2": np.ascontiguousarray(
            np.broadcast_to(b2_e.reshape(1, D), (P, D))),
    }


_NC_CACHE = None


def _get_nc():
    global _NC_CACHE
    if _NC_CACHE is None:
        _NC_CACHE = build_nc()
    return _NC_CACHE


def kernel(x, w1, b1, w2, b2, trace=False):
    x = np.asarray(x, dtype=np.float32)
    w1 = np.asarray(w1, dtype=np.float32)
    b1 = np.asarray(b1, dtype=np.float32)
    w2 = np.asarray(w2, dtype=np.float32)
    b2 = np.asarray(b2, dtype=np.float32)

    nc = _get_nc()
    in_maps = [
        make_in_map(x[:, e], w1[e], b1[e], w2[e], b2[e]) for e in range(N_CORES)
    ]
    res = run_bass_kernel_spmd(
        nc, in_maps, core_ids=list(range(N_CORES)), trace=trace)
    out = np.empty((B, E, N, D), np.float32)
    for e in range(N_CORES):
        out[:, e] = res.results[e]["out"].astype(np.float32).reshape(B, N, D)
    if trace:
        return out, res
    return out


# revision 63
# speedup vs baseline: 1.0046x; 1.0046x over previous
"""Grouped-experts MLP (MoE) kernel for Trainium2, expert-parallel over 8 cores.

Problem: x[B=2, E=8, N=1024, D=1024]; per expert e:
    out[:, e] = GELU(x[:, e] @ w1[e] + b1[e]) @ w2[e] + b2[e]
with w1[e]: [D=1024, H=4096], w2[e]: [H=4096, D=1024].

Sharding: expert axis across the 8 NeuronCores (core e owns expert e).
The host performs the "all-to-all": it hands core e the slab x[:, e]
plus expert e's weights and reassembles the full output afterward.

Per-core kernel (T = B*N = 2048 tokens), all matmul operands bf16 with
fp32 PSUM accumulation. The PE floor is 2048 matmuls x 512 cols
(~443us at ~2.4GHz); everything else is startup/tail/bubble control:

  - All inputs are host-packed into SBUF-image DRAM tensors [128, N]
    so every DMA is one large contiguous 2D slice (2KB+ rows nearly
    double aggregate DMA bandwidth vs 1KB rows). DMA completion
    follows issue order, and early aggregate bandwidth ramps from
    ~100 to ~400GB/s over the first ~8us - that ramp, not issue
    serialization, floors how fast the first MB can land.
  - To fill the ramp window the host also passes a duplicated
    256-token "head" slab of x-q0 (256KB). It and w1h0 land ~12.5us
    (the ramp floor), ~1-3us before the full 1MB x-q0, so the first 6
    layer-1 h-groups run as 256-column slab matmuls on the head
    tokens; their other halves follow immediately after (reusing
    w1h0-5, giving the w1 stream catch-up time before h6+).
  - An 11-matmul warm-up block of garbage matmuls spans the framework
    preamble end (~7.8us) to the first data (~12.5us). The HAM clock
    governor needs ~3.5us of GAPLESS PE activity to un-throttle
    1.2->2.4GHz (any sub-us stall resets the ramp while throttled;
    once full speed, brief stalls are safe - only multi-us idle
    re-throttles), so the warm block both finishes the clock ramp and
    bridges to the data with no gap.
  - w1 is packed h-major so each 256KB DMA feeds exactly one layer-1
    h-group; sync streams them in consumption order, then x-q1, w2,
    b2. Layer 1 computes hT[h][128, 512] per h-group (PSUM accum over
    8 D-tiles), GELU + b1 fused into the PSUM->SBUF eviction on the
    scalar engine, writing bf16.
  - Layer 2 accumulates 32 H-tiles of a [128 tok, 512 dcol] tile in
    PSUM, adds b2 on the DVE, DMAs out. The very last group runs as
    two 256-col accumulation sub-groups in separate PSUM tiles
    (sharing one tile makes B's start falsely wait on A's DVE read):
    half A's eviction + DMA overlap half B's matmuls, so after the
    last matmul only a 256-col DVE add and two parallel 32KB DMAs
    (sync + scalar; gpsimd's SWDGE drain is ~2.6us - never on the
    tail) remain before the fixed ~2.5us teardown. The output is
    written bf16 (host upcasts): halves out-DMA bytes and doubles the
    DVE eviction rate for ~1e-3 extra rel-err.
"""

import numpy as np
import ml_dtypes

import concourse.bacc as bacc
import concourse.mybir as mybir
import concourse.tile as tile
from concourse.bass_utils import run_bass_kernel_spmd

B, E, N, D, H = 2, 8, 1024, 1024, 4096
T = B * N          # tokens per expert
P = 128
N_CORES = 8

TQ = 512           # tokens per quarter
NQ = T // TQ       # 4
KD = D // P        # 8 k-tiles over D (layer-1 contraction)
KH = H // P        # 32 k-tiles over H (layer-2 contraction)
DC = D // 512      # 2 output column chunks
NTS = TQ // P      # 4 token subtiles per quarter
NSLAB = 6          # leading q0 h-groups computed as 256-token half slabs

F32 = mybir.dt.float32
BF16 = mybir.dt.bfloat16
GELU = mybir.ActivationFunctionType.Gelu
BF16_NP = ml_dtypes.bfloat16


def build_nc():
    nc = bacc.Bacc("TRN2", target_bir_lowering=False, debug=False)

    # Host-packed SBUF images: [128, cols]; column layout is the exact
    # SBUF layout, so each DMA is a contiguous 2D slice.
    #   xin:   [q][k][512]  (q*4096 + k*512)
    #   xhead: [k][256]     q0 tokens 0:255 duplicated (fast first slab)
    #   w1in:  [h][k][128]  (h*1024 + k*128)   h-major: DMA unit = h-group
    #   w2in:  [dc][k][512] (dc*16384 + k*512)
    xin = nc.dram_tensor("xin", [P, NQ * KD * TQ], BF16,
                         kind="ExternalInput")          # [128, 16384]
    xhead = nc.dram_tensor("xhead", [P, KD * 256], BF16,
                           kind="ExternalInput")        # [128, 2048]
    w1in = nc.dram_tensor("w1in", [P, D * H // P], BF16,
                          kind="ExternalInput")         # [128, 32768]
    w2in = nc.dram_tensor("w2in", [P, H * D // P], BF16,
                          kind="ExternalInput")         # [128, 32768]
    b1 = nc.dram_tensor("b1", [P, KH], F32, kind="ExternalInput")
    b2 = nc.dram_tensor("b2", [P, D], F32, kind="ExternalInput")
    # output in bf16: halves out-DMA bytes, doubles DVE eviction rate,
    # and shrinks the tail transfers; host upcasts to fp32 (adds at most
    # ~1.1e-3 to the rel-err, far inside the 2e-2 gate)
    out = nc.dram_tensor("out", [T, D], BF16, kind="ExternalOutput")
    scratch = nc.dram_tensor("scratch", [P, 8], BF16, kind="Internal")

    with tile.TileContext(nc) as tc:
        with (
            tc.tile_pool(name="const", bufs=1) as constp,
            tc.tile_pool(name="xp", bufs=2) as xp,
            tc.tile_pool(name="w1p", bufs=1) as w1p,
            tc.tile_pool(name="w2p", bufs=1) as w2p,
            tc.tile_pool(name="hTp", bufs=1) as hTp,
            tc.tile_pool(name="stp", bufs=4) as stp,
            tc.tile_pool(name="ps1p", bufs=5, space="PSUM") as ps1p,
            tc.tile_pool(name="ps2p", bufs=3, space="PSUM") as ps2p,
        ):
            def alloc_xq():
                return xp.tile([P, 4096], BF16, name="xq", tag="xq")

            def load_xq(eng, t, q):
                eng.dma_start(t[:], xin[:, q * 4096:(q + 1) * 4096])

            b1sb = constp.tile([P, KH], F32, name="b1sb")
            warm_src = constp.tile([P, 512], BF16, name="warm_src")
            xh = constp.tile([P, KD * 256], BF16, name="xh")
            xq0 = constp.tile([P, 4096], BF16, name="xq0")
            xq_tiles = [None] * NQ
            w1h = [w1p.tile([P, KD * P], BF16, name=f"w1h_{h}")
                   for h in range(KH)]

            def w1_st(h, k):
                return w1h[h][:, k * P:(k + 1) * P]

            def xh_mov(k):
                return xh[:, k * 256:(k + 1) * 256]

            # ---- startup DMAs, in consumption order across engines ----
            # Pre-gate traffic is exactly what the slab-A phase needs
            # (xh + w1h0-4 + b1, ~2.1MB). Everything else (xq0 first,
            # then the w1h stream, x-q1, w2, b2) waits on a gate DMA
            # that reads the first slab eviction, so it cannot crowd
            # the DMA FIFO during the bandwidth ramp. The gate must NOT
            # sit on scalar: it would deadlock against the ACTIVATE it
            # waits for.
            nc.sync.dma_start(w1h[0][:], w1in[:, 0:1024])
            nc.scalar.dma_start(xh[:], xhead[:])
            nc.gpsimd.memset(warm_src[:], 0.0)
            nc.gpsimd.dma_start(w1h[1][:], w1in[:, 1024:2048])
            nc.gpsimd.dma_start(b1sb[:], b1[:])
            nc.sync.dma_start(w1h[2][:], w1in[:, 2 * 1024:3 * 1024])
            nc.sync.dma_start(w1h[3][:], w1in[:, 3 * 1024:4 * 1024])
            nc.sync.dma_start(w1h[4][:], w1in[:, 4 * 1024:5 * 1024])
            nc.sync.dma_start(w1h[5][:], w1in[:, 5 * 1024:6 * 1024])
            w2t = {}
            b2sb = constp.tile([P, D], F32, name="b2sb")

            def emit_gated_loads(gate_tile):
                nc.sync.dma_start(scratch[:], gate_tile[:, 0:8])
                # xq0 leads the post-gate queue as TWO parallel 512KB
                # trains (a single 1MB train caps at ~220GB/s), so it
                # lands ~gate+2.7us — before the slab-B phase needs it
                nc.sync.dma_start(xq0[:, 0:2048], xin[:, 0:2048])
                nc.sync.dma_start(xq0[:, 2048:4096], xin[:, 2048:4096])
                for h in range(6, KH):
                    nc.sync.dma_start(
                        w1h[h][:], w1in[:, h * 1024:(h + 1) * 1024])
                xq_tiles[1] = alloc_xq()
                load_xq(nc.sync, xq_tiles[1], 1)
                # w2: 2 dc-halves, 4 tiles [128,4096] each (k-octet)
                for dc in range(DC):
                    for i in range(4):
                        t = w2p.tile([P, 4096], BF16, name=f"w2_{dc}_{i}")
                        nc.sync.dma_start(
                            t[:], w2in[:, dc * 16384 + i * 4096:
                                       dc * 16384 + (i + 1) * 4096])
                        w2t[(dc, i)] = t
                nc.sync.dma_start(b2sb[:], b2[:])

            # ---- HAM pre-warm ----
            warm_ps = ps2p.tile([P, 512], F32, name="warm_ps", tag="ps2")
            NWARM = 14
            for i in range(NWARM):
                nc.tensor.matmul(
                    warm_ps[:], warm_src[:, 0:P], warm_src[:],
                    start=(i == 0), stop=(i == NWARM - 1))

            def x_ap(q, xq, k):
                if q == 0:
                    return xq0[:, k * 512:(k + 1) * 512]
                return xq[:, k * 512:(k + 1) * 512]

            def w2_ap(dc, k):
                return w2t[(dc, k // 8)][:, (k % 8) * 512:(k % 8) * 512 + 512]

            def alloc_ht(h):
                return hTp.tile([P, TQ], BF16, name=f"hT_{h}", tag=f"hT_{h}")

            def l1_group(h, mov, ht, ht_sl):
                """One layer-1 h-group: 8 matmuls + fused GELU eviction.
                mov(k) gives the moving operand; ht_sl the hT col range."""
                w = ht_sl.stop - ht_sl.start
                ps = ps1p.tile([P, TQ], F32, name="ps1", tag="ps1")
                for k in range(KD):
                    nc.tensor.matmul(
                        ps[:, 0:w], w1_st(h, k), mov(k),
                        start=(k == 0), stop=(k == KD - 1))
                nc.scalar.activation(
                    ht[:, ht_sl], ps[:, 0:w], GELU, bias=b1sb[:, h:h + 1])

            for q in range(NQ):
                xq = xq_tiles[q]
                hTt = [None] * KH

                if q == 0:
                    # leading h-groups on the 256-token head slab (xh
                    # lands ~3us before the full x-q0)...
                    for h in range(NSLAB):
                        hTt[h] = alloc_ht(h)
                        l1_group(h, xh_mov, hTt[h], slice(0, 256))
                        if h == 0:
                            emit_gated_loads(hTt[0])
                    # ...then their trailing halves once x-q0 has landed
                    # (these reuse w1h0-5, giving sync's w1h stream ~5us
                    # of catch-up before h6+ needs fresh tiles)...
                    for h in range(NSLAB):
                        l1_group(
                            h, lambda k: xq0[:, k * 512 + 256:
                                             (k + 1) * 512],
                            hTt[h], slice(256, TQ))
                    # ...and the remaining full-width groups.
                    for h in range(NSLAB, KH):
                        hTt[h] = alloc_ht(h)
                        l1_group(
                            h, lambda k: x_ap(q, xq, k), hTt[h],
                            slice(0, TQ))
                else:
                    for h in range(KH):
                        hTt[h] = alloc_ht(h)
                        l1_group(
                            h, lambda k: x_ap(q, xq, k), hTt[h],
                            slice(0, TQ))

                # prefetch x for quarter q+2 (q/q+1 tiles both live;
                # pool bufs=2 recycles q's buffers once layer 1 is done)
                if q + 2 < NQ:
                    xq_tiles[q + 2] = alloc_xq()
                    load_xq(nc.sync, xq_tiles[q + 2], q + 2)

                # layer 2: out tile [128 tok, 512 dcol] accumulates all 32
                # H-tiles in PSUM, then +b2 on the DVE and straight to DRAM
                for dc in range(DC):
                    sl = slice(dc * 512, (dc + 1) * 512)
                    for ts in range(NTS):
                        t0 = q * TQ + ts * P
                        ps = ps2p.tile([P, 512], F32, name="ps2", tag="ps2")
                        last = (q == NQ - 1 and dc == DC - 1 and ts == NTS - 1)
                        if not last:
                            for k in range(KH):
                                nc.tensor.matmul(
                                    ps[:], hTt[k][:, ts * P:(ts + 1) * P],
                                    w2_ap(dc, k),
                                    start=(k == 0), stop=(k == KH - 1))
                            st = stp.tile([P, 512], BF16, name="st", tag="st")
                            nc.vector.tensor_add(st[:], b2sb[:, sl], ps[:])
                            nc.sync.dma_start(out[t0:t0 + P, sl], st[:])
                        else:
                            # tail: two 256-col sub-groups, separate PSUM
                            psB = ps2p.tile([P, 512], F32, name="ps2b",
                                            tag="ps2")
                            for hf in range(2):
                                pst = ps if hf == 0 else psB
                                c0 = dc * 512 + hf * 256
                                for k in range(KH):
                                    w2a = w2t[(dc, k // 8)][
                                        :, (k % 8) * 512 + hf * 256:
                                        (k % 8) * 512 + hf * 256 + 256]
                                    nc.tensor.matmul(
                                        pst[:, 0:256],
                                        hTt[k][:, ts * P:(ts + 1) * P],
                                        w2a,
                                        start=(k == 0), stop=(k == KH - 1))
                                stf = constp.tile([P, 256], BF16,
                                                  name=f"stf_{hf}")
                                nc.vector.tensor_add(
                                    stf[:], b2sb[:, c0:c0 + 256],
                                    pst[:, 0:256])
                                if hf == 0:
                                    nc.sync.dma_start(
                                        out[t0:t0 + P, c0:c0 + 256], stf[:])
                                else:
                                    nc.sync.dma_start(
                                        out[t0:t0 + P, c0:c0 + P],
                                        stf[:, 0:P])
                                    nc.scalar.dma_start(
                                        out[t0:t0 + P, c0 + P:c0 + 256],
                                        stf[:, P:256])

    nc.compile()
    return nc


def make_in_map(x_e, w1_e, b1_e, w2_e, b2_e):
    """Per-core input map: pack one expert's slabs into SBUF images."""
    xT = x_e.reshape(T, D).T.astype(BF16_NP)      # [D, T]
    xin = np.ascontiguousarray(
        xT.reshape(KD, P, NQ, TQ).transpose(1, 2, 0, 3).reshape(P, -1)
    )                                             # [128, q*4096 + k*512]
    xhead = np.ascontiguousarray(
        xT[:, 0:256].reshape(KD, P, 256).transpose(1, 0, 2).reshape(P, -1)
    )                                             # [128, k*256]
    w1in = np.ascontiguousarray(
        w1_e.reshape(KD, P, KH, P).transpose(1, 2, 0, 3).reshape(P, -1)
    ).astype(BF16_NP)                             # [128, h*1024 + k*128]
    w2in = np.ascontiguousarray(
        w2_e.reshape(KH, P, DC, 512).transpose(1, 2, 0, 3).reshape(P, -1)
    ).astype(BF16_NP)                             # [128, dc*16384 + k*512]
    return {
        "xin": xin,
        "xhead": xhead,
        "w1in": w1in,
        "w2in": w2in,
        "b1": np.ascontiguousarray(b1_e.reshape(KH, P).T),
        "b2": np.ascontiguousarray(
            np.broadcast_to(b2_e.reshape(1, D), (P, D))),
    }


_NC_CACHE = None


def _get_nc():
    global _NC_CACHE
    if _NC_CACHE is None:
        _NC_CACHE = build_nc()
    return _NC_CACHE


def kernel(x, w1, b1, w2, b2, trace=False):
    x = np.asarray(x, dtype=np.float32)
    w1 = np.asarray(w1, dtype=np.float32)
    b1 = np.asarray(b1, dtype=np.float32)
    w2 = np.asarray(w2, dtype=np.float32)
    b2 = np.asarray(b2, dtype=np.float32)

    nc = _get_nc()
    in_maps = [
        make_in_map(x[:, e], w1[e], b1[e], w2[e], b2[e]) for e in range(N_CORES)
    ]
    res = run_bass_kernel_spmd(
        nc, in_maps, core_ids=list(range(N_CORES)), trace=trace)
    out = np.empty((B, E, N, D), np.float32)
    for e in range(N_CORES):
        out[:, e] = res.results[e]["out"].astype(np.float32).reshape(B, N, D)
    if trace:
        return out, res
    return out
